# revision 14
# baseline (speedup 1.0000x reference)
"""Trainium2 Bass kernel for nn_Attention_3D (channel attention / XCA-style 3D module).

Reference computation:
  qkv = 1x1x1 conv (pointwise, 64->192ch) -> depthwise 3x3x3 conv (SAME pad)
  q,k,v = split(qkv); q,k l2-normalized over the full spatial dim n = d*h*w
  attn = softmax_e(q_hat @ k_hat^T * temperature)  per (batch, head) -> [8x8]
  out = attn @ v  -> 1x1x1 projection (64->64ch)

Sharding: spatial over h (128 rows -> 16 per core, halo +-1 for the depthwise
conv). Each core holds ALL channels of its h-slab, so the only cross-core
data are the tiny per-(batch) Gram matrices G = [q|k]^T [q|k] (two 128x128
fp32 = 128KB) which are summed with one AllReduce. l2 norms come from diag(G),
attention weights are computed on-chip, and the projection needs no collective.
"""

import numpy as np
import ml_dtypes

import concourse.bass as bass
import concourse.mybir as mybir
import concourse.tile as tile
import concourse.bacc as bacc
from concourse import bass_utils

F32 = mybir.dt.float32
F32R = mybir.dt.float32r
BF16 = mybir.dt.bfloat16
FP8 = mybir.dt.float8e4

B, C, D, H, W = 2, 64, 8, 128, 128
HEADS, CH = 8, 8
C3 = 3 * C                      # 192 qkv channels
N_CORES = 8
HL = H // N_CORES               # 16 local output h rows
HLH = HL + 2                    # 18 rows with halo
WP = W + 2                      # 130 padded w
MS = HLH * W                    # 2304 pointwise cols per d-slice
MP = HLH * WP                   # 2340 padded qkv cols per d-slice
MOUT = D * HL * W               # 16384 output cols per partition row
EPS = 1e-12

# tap order: t = kd*9 + kh*3 + kw, shifts are indices into padded coords
TAPS = [(kd, kh, kw) for kd in range(3) for kh in range(3) for kw in range(3)]

_CACHE = {}


def _prep_consts(w_qkv, w_dw, temperature, w_proj):
    """Host-side constant tensors shared by all cores."""
    w_qkv = np.asarray(w_qkv, np.float32)
    w_dw = np.asarray(w_dw, np.float32)
    temp = np.asarray(temperature, np.float32).reshape(HEADS)
    w_proj = np.asarray(w_proj, np.float32)

    # Pointwise weights, block-diagonal over batch.
    # out rows o2 = beta*128+j; contraction rows k = (b, c).
    # beta0 = (b0, qkv rows 0..127 (q|k)), beta1 = (b1, rows 0..127),
    # beta2 = (b0, v rows 128..191 | b1, v rows 128..191)
    W2 = np.zeros((128, 384), np.float32)
    W2[0:64, 0:128] = w_qkv[0:128].T
    W2[64:128, 128:256] = w_qkv[0:128].T
    W2[0:64, 256:320] = w_qkv[128:192].T
    W2[64:128, 320:384] = w_qkv[128:192].T

    # Depthwise diagonal matrices.
    # v group (bf16): rows <-> w_dw rows 128..192 duplicated for both batches.
    # qk group (fp8): rows <-> w_dw rows 0..127; taps paired per kd for
    # DoubleRow: j = kh*3+kw pairs (0,1),(2,3),(4,5),(6,7) + single j=8.
    wd = w_dw.reshape(C3, 27)
    vv = np.concatenate([wd[128:192], wd[128:192]], 0)   # [128, 27]
    qk = wd[0:128]                                       # [128, 27]
    idx = np.arange(128)
    wdv = np.zeros((27, 128, 128), np.float32)
    for t in range(27):
        wdv[t, idx, idx] = vv[:, t]
    wdv = wdv.astype(ml_dtypes.bfloat16)
    f8 = mybir.dt.np(mybir.dt.float8e4)
    wdqk_pr = np.zeros((12, 128, 2, 128), np.float32)
    wdqk_sg = np.zeros((3, 128, 128), np.float32)
    for kd in range(3):
        for p in range(4):
            for i in range(2):
                t = kd * 9 + 2 * p + i
                wdqk_pr[kd * 4 + p, idx, i, idx] = qk[:, t]
        wdqk_sg[kd, idx, idx] = qk[:, kd * 9 + 8]
    wdqk_pr = wdqk_pr.astype(f8)
    wdqk_sg = wdqk_sg.astype(f8)
    wv_sc = np.ascontiguousarray(vv.astype(np.float32))   # [128, 27]
    ident8 = np.eye(128, dtype=np.float32).astype(f8)

    # Projection lhsT, block-diagonal over batch: lhsT[cin, o] = w_proj[o, cin]
    Wp2 = np.zeros((128, 128), np.float32)
    Wp2[0:64, 0:64] = w_proj.T
    Wp2[64:128, 64:128] = w_proj.T

    # temperature per q-row (rows 0..63 = 8h+c), 1.0 on k-rows
    tvec = np.ones((128, 1), np.float32)
    tvec[0:64, 0] = np.repeat(temp, CH)

    # block-diagonal mask of the q-k quadrant
    bdmask = np.zeros((128, 128), np.float32)
    for h in range(HEADS):
        bdmask[8 * h:8 * h + 8, 64 + 8 * h:64 + 8 * h + 8] = 1.0

    ident = np.eye(128, dtype=np.float32)
    ones_row = np.ones((1, 128), np.float32)
    return dict(w2=W2, wdv=wdv, wdqk_pr=wdqk_pr, wdqk_sg=wdqk_sg,
                ident8=ident8, wv_sc=wv_sc, wp2=Wp2, tvec=tvec, bdmask=bdmask,
                ident=ident, ones_row=ones_row)


def _prep_x_shards(x):
    """Per-core x slabs [128=(b,c), D, HLH*W] with zero h-halo at edges."""
    x = np.asarray(x, np.float32)
    shards = []
    for r in range(N_CORES):
        slab = np.zeros((B, C, D, HLH, W), np.float32)
        h0, h1 = 16 * r - 1, 16 * r + 17
        s0, s1 = max(h0, 0), min(h1, H)
        slab[:, :, :, s0 - h0:s1 - h0, :] = x[:, :, :, s0:s1, :]
        shards.append(np.ascontiguousarray(slab.reshape(128, D, MS)))
    return shards


def _build_program():
    nc = bacc.Bacc("TRN2", target_bir_lowering=False, debug=False,
                   num_devices=N_CORES)

    x_d = nc.dram_tensor("x_sh", [128, D, MS], F32R, kind="ExternalInput").ap()
    w2_d = nc.dram_tensor("w2", [128, 384], F32R, kind="ExternalInput").ap()
    wdv_d = nc.dram_tensor("wdv", [27, 128, 128], BF16,
                           kind="ExternalInput").ap()
    wqp_d = nc.dram_tensor("wdqk_pr", [12, 128, 2, 128], FP8,
                           kind="ExternalInput").ap()
    wqs_d = nc.dram_tensor("wdqk_sg", [3, 128, 128], FP8,
                           kind="ExternalInput").ap()
    id8_d = nc.dram_tensor("ident8", [128, 128], FP8,
                           kind="ExternalInput").ap()
    wvsc_d = nc.dram_tensor("wv_sc", [128, 27], F32,
                            kind="ExternalInput").ap()
    wp2_d = nc.dram_tensor("wp2", [128, 128], F32R, kind="ExternalInput").ap()
    tvec_d = nc.dram_tensor("tvec", [128, 1], F32, kind="ExternalInput").ap()
    bdmask_d = nc.dram_tensor("bdmask", [128, 128], F32, kind="ExternalInput").ap()
    ident_d = nc.dram_tensor("ident", [128, 128], F32, kind="ExternalInput").ap()
    identr_d = nc.dram_tensor("ident_r", [128, 128], F32R, kind="ExternalInput").ap()
    ones_d = nc.dram_tensor("ones_row", [1, 128], F32R, kind="ExternalInput").ap()
    zeros_d = nc.dram_tensor("zeros128", [128, 128], F32R, kind="ExternalInput").ap()

    out_d = nc.dram_tensor("out_sh", [128, MOUT], F32, kind="ExternalOutput").ap()
    dbg_g = nc.dram_tensor("dbg_g", [128, 256], F32, kind="ExternalOutput").ap()
    dbg_ab = nc.dram_tensor("dbg_ab", [128, 128], F32R, kind="ExternalOutput").ap()

    with tile.TileContext(nc) as tc:
        _emit(nc, tc, x_d, w2_d, wdv_d, wqp_d, wqs_d, id8_d, wvsc_d, wp2_d,
              tvec_d, bdmask_d, ident_d, identr_d, ones_d, zeros_d, out_d,
              dbg_g, dbg_ab)
    nc.compile()
    return nc


def _emit(nc, tc, x_d, w2_d, wdv_d, wqp_d, wqs_d, id8_d, wvsc_d, wp2_d,
          tvec_d, bdmask_d, ident_d, identr_d, ones_d, zeros_d, out_d,
          dbg_g, dbg_ab):
    from contextlib import ExitStack
    es = ExitStack()

    cons = es.enter_context(tc.tile_pool(name="cons", bufs=1))
    xp = es.enter_context(tc.tile_pool(name="xp", bufs=3))
    qkvp = es.enter_context(tc.tile_pool(name="qkvp", bufs=4))
    vslp = es.enter_context(tc.tile_pool(name="vslp", bufs=5))
    vaccp = es.enter_context(tc.tile_pool(name="vaccp", bufs=3))
    vp = es.enter_context(tc.tile_pool(name="vp", bufs=1))
    stp = es.enter_context(tc.tile_pool(name="stp", bufs=3))
    qtp = es.enter_context(tc.tile_pool(name="qtp", bufs=3))
    gsp = es.enter_context(tc.tile_pool(name="gsp", bufs=1))
    smp = es.enter_context(tc.tile_pool(name="smp", bufs=1))
    outp = es.enter_context(tc.tile_pool(name="outp", bufs=2))
    dramp = es.enter_context(tc.tile_pool(name="dramp", bufs=1, space="DRAM"))

    pw_ps = es.enter_context(tc.tile_pool(name="pw_ps", bufs=2, space="PSUM"))
    dw_ps = es.enter_context(tc.tile_pool(name="dw_ps", bufs=2, space="PSUM"))
    tr_ps = es.enter_context(tc.tile_pool(name="tr_ps", bufs=2, space="PSUM"))
    gr_ps = es.enter_context(tc.tile_pool(name="gr_ps", bufs=2, space="PSUM"))

    # ---- constants ----
    w2s = cons.tile([128, 384], F32R, tag="w2s")
    nc.sync.dma_start(w2s[:], w2_d[:])
    wdvs = cons.tile([128, 27 * 128], BF16, tag="wdvs")
    wdvv = wdvs[:].rearrange("p (t j) -> p t j", t=27, j=128)
    nc.sync.dma_start(wdvv, wdv_d.rearrange("t i j -> i t j"))
    wqp = cons.tile([128, 12 * 2 * 128], FP8, tag="wqp")
    wqpv = wqp[:].rearrange("p (pr i j) -> p pr i j", pr=12, i=2, j=128)
    nc.sync.dma_start(wqpv, wqp_d.rearrange("pr i a j -> i pr a j"))
    wqs = cons.tile([128, 3 * 128], FP8, tag="wqs")
    wqsv = wqs[:].rearrange("p (t j) -> p t j", t=3, j=128)
    nc.sync.dma_start(wqsv, wqs_d.rearrange("t i j -> i t j"))
    wvs = cons.tile([128, 27], F32, tag="wvs")
    nc.sync.dma_start(wvs[:], wvsc_d[:])
    idb = cons.tile([128, 128], BF16, tag="idb")
    from concourse.masks import make_identity
    make_identity(nc, idb[:])
    wp2s = cons.tile([128, 128], F32R, tag="wp2s")
    nc.sync.dma_start(wp2s[:], wp2_d[:])
    tvs = cons.tile([128, 1], F32, tag="tvs")
    nc.sync.dma_start(tvs[:], tvec_d[:])
    bds = cons.tile([128, 128], F32, tag="bds")
    nc.sync.dma_start(bds[:], bdmask_d[:])
    ids = cons.tile([128, 128], F32, tag="ids")
    nc.sync.dma_start(ids[:], ident_d[:])
    idr = cons.tile([128, 128], F32R, tag="idr")
    nc.sync.dma_start(idr[:], identr_d[:])
    on1 = cons.tile([1, 128], F32R, tag="on1")
    nc.sync.dma_start(on1[:], ones_d[:])
    zqv = cons.tile([128, MP], BF16, tag="zqv")
    nc.gpsimd.memset(zqv[:], 0.0)
    zqk = cons.tile([128, MP], FP8, tag="zqk")
    nc.gpsimd.memset(zqk[:], 0.0)

    g_sb = []
    for b in range(2):
        g = gsp.tile([128, 128], F32, tag=f"g{b}")
        nc.vector.memset(g[:], 0.0)
        g_sb.append(g)

    vres = vp.tile([128, MOUT], F32R, tag="vres")

    def evac(dst, src):
        nc.scalar.copy(dst, src)

    # ---- phase 1: pointwise -> depthwise -> gram, software-pipelined over
    # d: step s runs pointwise(s), qk-depthwise(s-1), v-depthwise(s-2). The
    # two trailing v iterations keep the PE busy under the gram AllReduce.
    qk_slots = [None] * D
    v_slots = [None] * D

    def tap_ap(tile_ap, offset, dims):
        a = tile_ap.copy()
        pstride = list(a.ap)[0][0]
        a.ap = mybir.VecI64Pair([[pstride, 128]] + dims)
        a.offset = offset
        return a

    def pointwise(d):
        xs = xp.tile([128, MS], F32R, tag="xs")
        nc.sync.dma_start(xs[:], x_d[:, d])
        qks = qkvp.tile([128, 2 * MP], FP8, tag="qk")
        vs = vslp.tile([128, MP], BF16, tag="vsl")
        qk_slots[d] = qks
        v_slots[d] = vs
        qksv = qks[:].rearrange("p (beta hh ww) -> p beta hh ww",
                                beta=2, hh=HLH, ww=WP)
        vsv = vs[:].rearrange("p (hh ww) -> p hh ww", hh=HLH, ww=WP)
        # zero padded w-border columns (slots rotate; memory is dirty)
        nc.gpsimd.memset(qksv[:, :, :, 0:WP:WP - 1], 0.0)
        nc.gpsimd.memset(vsv[:, :, 0:WP:WP - 1], 0.0)
        for beta in range(3):
            for t6 in range(6):
                ps = pw_ps.tile([128, 384], F32, tag="pw")
                nc.tensor.matmul(
                    ps[:],
                    w2s[:, 128 * beta:128 * (beta + 1)],
                    xs[:, 384 * t6:384 * (t6 + 1)],
                    start=True, stop=True)
                if beta < 2:
                    dst = qksv[:, beta, 3 * t6:3 * t6 + 3, 1:1 + W]
                else:
                    dst = vsv[:, 3 * t6:3 * t6 + 3, 1:1 + W]
                evac(dst, ps[:])

    # per-kd tap pairing: j = kh*3+kw; pairs (0,1),(2,3),(4,5),(6,7), single 8
    PAIR_J0 = [0, 2, 4, 6]
    J_OFF = [kh * WP + kw for kh in range(3) for kw in range(3)]

    def qk_dw(do):
        for beta in range(2):
            gp = gr_ps.tile([128, 128], F32, tag="gram")
            n_gmm = 0
            for t4 in range(4):
                dps = dw_ps.tile([128, 512], F32, tag="dw")
                mm = 0
                for kd in range(3):
                    dd = do - 1 + kd
                    slot = qk_slots[dd] if 0 <= dd < D else None
                    base = (beta * MP if slot is not None else 0) + 4 * t4 * WP
                    src = slot if slot is not None else zqk
                    for j0 in PAIR_J0:
                        delta = J_OFF[j0 + 1] - J_OFF[j0]
                        rhs = tap_ap(src[:], base + J_OFF[j0],
                                     [[delta, 2], [WP, 4], [1, W]])
                        nc.tensor.matmul(
                            dps[:], wqpv[:, 4 * kd + j0 // 2], rhs,
                            start=(mm == 0), stop=False,
                            perf_mode=mybir.MatmulPerfMode.DoubleRow)
                        mm += 1
                    rhs = tap_ap(src[:], base + J_OFF[8],
                                 [[WP, 4], [1, W]])
                    nc.tensor.matmul(dps[:], wqsv[:, kd], rhs,
                                     start=False, stop=(kd == 2))
                    mm += 1
                st = stp.tile([128, 512], BF16, tag="st")
                evac(st[:], dps[:])
                for ch4 in range(4):
                    trp = tr_ps.tile([128, 128], BF16, tag="tr")
                    nc.tensor.transpose(
                        trp[:], st[:, 128 * ch4:128 * (ch4 + 1)], idb[:])
                    qt = qtp.tile([128, 128], BF16, tag="qt")
                    evac(qt[:], trp[:])
                    nc.tensor.matmul(gp[:], qt[:], qt[:],
                                     start=(n_gmm == 0), stop=(n_gmm == 15))
                    n_gmm += 1
            nc.vector.tensor_add(g_sb[beta][:], g_sb[beta][:], gp[:])

    def v_dw(do):
        for t4 in range(4):
            dps = dw_ps.tile([128, 512], F32, tag="dw")
            acc = vaccp.tile([128, 512], mybir.dt.float16, tag="vacc")
            pe_i = dve_i = 0
            for t, (kd, kh, kw) in enumerate(TAPS):
                dd = do - 1 + kd
                src = v_slots[dd] if 0 <= dd < D else zqv
                sv = src[:].rearrange("p (hh ww) -> p hh ww", hh=HLH, ww=WP)
                rhs = sv[:, 4 * t4 + kh:4 * t4 + kh + 4, kw:kw + W]
                if kw == 1:
                    nc.tensor.matmul(dps[:], wdvv[:, t], rhs,
                                     start=(pe_i == 0), stop=(pe_i == 8))
                    pe_i += 1
                else:
                    sc = wvs[:, t:t + 1]
                    if dve_i == 0:
                        nc.vector.tensor_scalar_mul(acc[:], rhs, sc)
                    else:
                        nc.vector.scalar_tensor_tensor(
                            acc[:], rhs, sc, acc[:],
                            mybir.AluOpType.mult, mybir.AluOpType.add)
                    dve_i += 1
            nc.vector.tensor_add(
                vres[:, 2048 * do + 512 * t4:2048 * do + 512 * (t4 + 1)],
                dps[:], acc[:])

    for step in range(D + 3):
        if step < D:
            pointwise(step)
        if 0 <= step - 1 < D:
            qk_dw(step - 1)
        if 0 <= step - 3 < D:
            v_dw(step - 3)

    # ---- all-reduce the grams ----
    bnc_in = dramp.tile([128, 256], F32, tag="bnc_in")
    bnc_out = dramp.tile([128, 256], F32, tag="bnc_out", addr_space="Shared")
    nc.gpsimd.dma_start(bnc_in[:, 0:128], g_sb[0][:])
    nc.gpsimd.dma_start(bnc_in[:, 128:256], g_sb[1][:])
    nc.gpsimd.collective_compute(
        "AllReduce", mybir.AluOpType.add,
        replica_groups=[list(range(N_CORES))],
        ins=[bnc_in.opt()], outs=[bnc_out.opt()])
    nc.gpsimd.dma_start(g_sb[0][:], bnc_out[:, 0:128])
    nc.gpsimd.dma_start(g_sb[1][:], bnc_out[:, 128:256])
    nc.sync.dma_start(dbg_g[:, 0:128], g_sb[0][:])
    nc.sync.dma_start(dbg_g[:, 128:256], g_sb[1][:])

    # ---- softmax -> attention weights AB (block-diag per batch) ----
    ab = smp.tile([128, 128], F32R, tag="ab")
    nc.sync.dma_start(ab[:], zeros_d[:])
    for b in range(2):
        gb = g_sb[b]
        dtmp = smp.tile([128, 128], F32, tag=f"dtmp{b}")
        nc.vector.tensor_mul(dtmp[:], gb[:], ids[:])
        dvec = smp.tile([128, 1], F32, tag=f"dvec{b}")
        nc.vector.reduce_sum(dvec[:], dtmp[:], axis=mybir.AxisListType.X)
        nrm = smp.tile([128, 1], F32, tag=f"nrm{b}")
        nc.scalar.activation(nrm[:], dvec[:], mybir.ActivationFunctionType.Sqrt)
        nc.vector.tensor_scalar_max(nrm[:], nrm[:], EPS)
        rn = smp.tile([128, 1], F32, tag=f"rn{b}")
        nc.vector.reciprocal(rn[:], nrm[:])
        # RK[p, j] = rn[j] via transpose + K=1 outer product
        rtp = tr_ps.tile([1, 128], F32, tag="tr")
        nc.tensor.transpose(rtp[:], rn[:], ids[:])
        rnt_row = smp.tile([1, 128], F32R, tag=f"rnt_row{b}")
        nc.vector.tensor_copy(rnt_row[:], rtp[:])
        rkp = gr_ps.tile([128, 128], F32, tag="gram")
        nc.tensor.matmul(rkp[:], on1[:], rnt_row[:],
                         start=True, stop=True)
        rk = smp.tile([128, 128], F32, tag=f"rk{b}")
        nc.vector.tensor_copy(rk[:], rkp[:])
        # s = G * (rn*temp)[row] * rn[col]
        rnt = smp.tile([128, 1], F32, tag=f"rnt{b}")
        nc.vector.tensor_mul(rnt[:], rn[:], tvs[:])
        s1 = smp.tile([128, 128], F32, tag=f"s1{b}")
        nc.vector.tensor_scalar_mul(s1[:], gb[:], rnt[:])
        nc.vector.tensor_mul(s1[:], s1[:], rk[:])
        e = smp.tile([128, 128], F32, tag=f"e{b}")
        nc.scalar.activation(e[:], s1[:], mybir.ActivationFunctionType.Exp)
        nc.vector.tensor_mul(e[:], e[:], bds[:])
        ssum = smp.tile([128, 1], F32, tag=f"ssum{b}")
        nc.vector.reduce_sum(ssum[:], e[:, 64:128], axis=mybir.AxisListType.X)
        nc.vector.tensor_scalar_max(ssum[:], ssum[:], 1e-30)
        rs = smp.tile([128, 1], F32, tag=f"rs{b}")
        nc.vector.reciprocal(rs[:], ssum[:])
        apre = smp.tile([128, 128], F32, tag=f"apre{b}")
        nc.vector.tensor_scalar_mul(apre[:], e[:], rs[:])
        # transpose the q-k quadrant into the (b,b) diagonal block of AB
        abp = tr_ps.tile([64, 64], F32, tag="tr")
        nc.tensor.transpose(abp[:], apre[0:64, 64:128], ids[0:64, 0:64])
        if b == 0:
            nc.vector.tensor_copy(ab[0:64, 0:64], abp[:])
        else:
            absb = smp.tile([64, 64], F32R, tag="absb")
            nc.vector.tensor_copy(absb[:], abp[:])
            nc.sync.dma_start(ab[64:128, 64:128], absb[:])
    nc.sync.dma_start(dbg_ab[:], ab[:])

    # ---- attn @ v -> projection -> out ----
    for mt in range(MOUT // 512):
        sl = slice(512 * mt, 512 * (mt + 1))
        avp = dw_ps.tile([128, 512], F32, tag="dw")
        nc.tensor.matmul(avp[:], ab[:], vres[:, sl],
                         start=True, stop=True)
        avs = stp.tile([128, 512], F32R, tag="st")
        evac(avs[:], avp[:])
        prp = dw_ps.tile([128, 512], F32, tag="dw")
        nc.tensor.matmul(prp[:], wp2s[:], avs[:],
                         start=True, stop=True)
        ots = outp.tile([128, 512], F32, tag="ots")
        evac(ots[:], prp[:])
        nc.sync.dma_start(out_d[:, sl], ots[:])

    es.close()


def _prep_in_maps(x, w_qkv, w_dw, temperature, w_proj):
    consts = _prep_consts(w_qkv, w_dw, temperature, w_proj)
    shards = _prep_x_shards(x)
    in_maps = []
    for r in range(N_CORES):
        in_maps.append({
            "x_sh": shards[r],
            "w2": consts["w2"],
            "wdv": consts["wdv"],
            "wdqk_pr": consts["wdqk_pr"],
            "wdqk_sg": consts["wdqk_sg"],
            "ident8": consts["ident8"],
            "wv_sc": consts["wv_sc"],
            "wp2": consts["wp2"],
            "tvec": consts["tvec"],
            "bdmask": consts["bdmask"],
            "ident": consts["ident"],
            "ident_r": consts["ident"],
            "ones_row": consts["ones_row"],
            "zeros128": np.zeros((128, 128), np.float32),
        })
    return in_maps


def _unshard(res):
    out = np.empty((B, C, D, H, W), np.float32)
    for r in range(N_CORES):
        slab = res.results[r]["out_sh"].reshape(B, C, D, HL, W)
        out[:, :, :, HL * r:HL * (r + 1), :] = slab
    return out


def kernel(x, w_qkv, w_dw, temperature, w_proj):
    if "nc" not in _CACHE:
        _CACHE["nc"] = _build_program()
    in_maps = _prep_in_maps(x, w_qkv, w_dw, temperature, w_proj)
    res = bass_utils.run_bass_kernel_spmd(
        _CACHE["nc"], in_maps, core_ids=list(range(N_CORES)))
    _CACHE["last_res"] = res
    return _unshard(res)


def run_profiled(x, w_qkv, w_dw, temperature, w_proj, **trace_kw):
    if "nc" not in _CACHE:
        _CACHE["nc"] = _build_program()
    in_maps = _prep_in_maps(x, w_qkv, w_dw, temperature, w_proj)
    res = bass_utils.run_bass_kernel_spmd(
        _CACHE["nc"], in_maps, core_ids=list(range(N_CORES)),
        trace=True, trace_cores=list(range(N_CORES)), **trace_kw)
    _CACHE["last_res"] = res
    return res


if __name__ == "__main__":
    rng = np.random.default_rng(0)
    x = rng.standard_normal((B, C, D, H, W), dtype=np.float32)
    w_qkv = rng.standard_normal((C3, C), dtype=np.float32) * 0.05
    w_dw = rng.standard_normal((C3, 1, 3, 3, 3), dtype=np.float32) * 0.05
    temperature = np.ones((HEADS, 1, 1), np.float32)
    w_proj = rng.standard_normal((C, C), dtype=np.float32) * 0.05
    out = kernel(x=x, w_qkv=w_qkv, w_dw=w_dw, temperature=temperature,
                 w_proj=w_proj)
    print("out", out.shape, out.dtype, np.abs(out).mean())


# revision 16
# speedup vs baseline: 1.1021x; 1.1021x over previous
"""Trainium2 Bass kernel for nn_Attention_3D (channel attention / XCA-style 3D module).

Reference computation:
  qkv = 1x1x1 conv (pointwise, 64->192ch) -> depthwise 3x3x3 conv (SAME pad)
  q,k,v = split(qkv); q,k l2-normalized over the full spatial dim n = d*h*w
  attn = softmax_e(q_hat @ k_hat^T * temperature)  per (batch, head) -> [8x8]
  out = attn @ v  -> 1x1x1 projection (64->64ch)

Sharding: spatial over h (128 rows -> 16 per core, halo +-1 for the depthwise
conv). Each core holds ALL channels of its h-slab, so the only cross-core
data are the tiny per-(batch) Gram matrices G = [q|k]^T [q|k] (two 128x128
fp32 = 128KB) which are summed with one AllReduce. l2 norms come from diag(G),
attention weights are computed on-chip, and the projection needs no collective.
"""

import numpy as np
import ml_dtypes

import concourse.bass as bass
import concourse.mybir as mybir
import concourse.tile as tile
import concourse.bacc as bacc
from concourse import bass_utils

F32 = mybir.dt.float32
F32R = mybir.dt.float32r
BF16 = mybir.dt.bfloat16
FP16 = mybir.dt.float16
FP8 = mybir.dt.float8e4

B, C, D, H, W = 2, 64, 8, 128, 128
HEADS, CH = 8, 8
C3 = 3 * C                      # 192 qkv channels
N_CORES = 8
HL = H // N_CORES               # 16 local output h rows
HLH = HL + 2                    # 18 rows with halo
WP = W + 2                      # 130 padded w
MS = HLH * W                    # 2304 pointwise cols per d-slice
MP = HLH * WP                   # 2340 padded qkv cols per d-slice
MOUT = D * HL * W               # 16384 output cols per partition row
EPS = 1e-12

# tap order: t = kd*9 + kh*3 + kw, shifts are indices into padded coords
TAPS = [(kd, kh, kw) for kd in range(3) for kh in range(3) for kw in range(3)]

_CACHE = {}


def _prep_consts(w_qkv, w_dw, temperature, w_proj):
    """Host-side constant tensors shared by all cores."""
    w_qkv = np.asarray(w_qkv, np.float32)
    w_dw = np.asarray(w_dw, np.float32)
    temp = np.asarray(temperature, np.float32).reshape(HEADS)
    w_proj = np.asarray(w_proj, np.float32)

    # Pointwise weights, block-diagonal over batch.
    # out rows o2 = beta*128+j; contraction rows k = (b, c).
    # beta0 = (b0, qkv rows 0..127 (q|k)), beta1 = (b1, rows 0..127),
    # beta2 = (b0, v rows 128..191 | b1, v rows 128..191)
    W2 = np.zeros((128, 384), np.float32)
    W2[0:64, 0:128] = w_qkv[0:128].T
    W2[64:128, 128:256] = w_qkv[0:128].T
    W2[0:64, 256:320] = w_qkv[128:192].T
    W2[64:128, 320:384] = w_qkv[128:192].T

    # Depthwise diagonal matrices.
    # v group (bf16): rows <-> w_dw rows 128..192 duplicated for both batches.
    # qk group (fp8): rows <-> w_dw rows 0..127; taps paired per kd for
    # DoubleRow: j = kh*3+kw pairs (0,1),(2,3),(4,5),(6,7) + single j=8.
    wd = w_dw.reshape(C3, 27)
    vv = np.concatenate([wd[128:192], wd[128:192]], 0)   # [128, 27]
    qk = wd[0:128]                                       # [128, 27]
    idx = np.arange(128)
    wdv = np.zeros((27, 128, 128), np.float32)
    for t in range(27):
        wdv[t, idx, idx] = vv[:, t]
    wdv = wdv.astype(np.float16)
    f8 = mybir.dt.np(mybir.dt.float8e4)
    wdqk_pr = np.zeros((12, 128, 2, 128), np.float32)
    wdqk_sg = np.zeros((3, 128, 128), np.float32)
    for kd in range(3):
        for p in range(4):
            for i in range(2):
                t = kd * 9 + 2 * p + i
                wdqk_pr[kd * 4 + p, idx, i, idx] = qk[:, t]
        wdqk_sg[kd, idx, idx] = qk[:, kd * 9 + 8]
    wdqk_pr = wdqk_pr.astype(f8)
    wdqk_sg = wdqk_sg.astype(f8)
    wv_sc = np.ascontiguousarray(vv.astype(np.float32))   # [128, 27]
    ident8 = np.eye(128, dtype=np.float32).astype(f8)

    # Projection lhsT, block-diagonal over batch: lhsT[cin, o] = w_proj[o, cin]
    Wp2 = np.zeros((128, 128), np.float32)
    Wp2[0:64, 0:64] = w_proj.T
    Wp2[64:128, 64:128] = w_proj.T

    # temperature per q-row (rows 0..63 = 8h+c), 1.0 on k-rows
    tvec = np.ones((128, 1), np.float32)
    tvec[0:64, 0] = np.repeat(temp, CH)

    # block-diagonal mask of the q-k quadrant
    bdmask = np.zeros((128, 128), np.float32)
    for h in range(HEADS):
        bdmask[8 * h:8 * h + 8, 64 + 8 * h:64 + 8 * h + 8] = 1.0

    ident = np.eye(128, dtype=np.float32)
    ones_row = np.ones((1, 128), np.float32)
    return dict(w2=W2, wdv=wdv, wdqk_pr=wdqk_pr, wdqk_sg=wdqk_sg,
                ident8=ident8, wv_sc=wv_sc, wp2=Wp2, tvec=tvec, bdmask=bdmask,
                ident=ident, ones_row=ones_row)


def _prep_x_shards(x):
    """Per-core x slabs [128=(b,c), D, HLH*W] with zero h-halo at edges."""
    x = np.asarray(x, np.float32)
    shards = []
    for r in range(N_CORES):
        slab = np.zeros((B, C, D, HLH, W), np.float32)
        h0, h1 = 16 * r - 1, 16 * r + 17
        s0, s1 = max(h0, 0), min(h1, H)
        slab[:, :, :, s0 - h0:s1 - h0, :] = x[:, :, :, s0:s1, :]
        shards.append(np.ascontiguousarray(slab.reshape(128, D, MS)))
    return shards


def _build_program():
    nc = bacc.Bacc("TRN2", target_bir_lowering=False, debug=False,
                   num_devices=N_CORES)

    x_d = nc.dram_tensor("x_sh", [128, D, MS], F32R, kind="ExternalInput").ap()
    w2_d = nc.dram_tensor("w2", [128, 384], F32R, kind="ExternalInput").ap()
    wdv_d = nc.dram_tensor("wdv", [27, 128, 128], FP16,
                           kind="ExternalInput").ap()
    wqp_d = nc.dram_tensor("wdqk_pr", [12, 128, 2, 128], FP8,
                           kind="ExternalInput").ap()
    wqs_d = nc.dram_tensor("wdqk_sg", [3, 128, 128], FP8,
                           kind="ExternalInput").ap()
    id8_d = nc.dram_tensor("ident8", [128, 128], FP8,
                           kind="ExternalInput").ap()
    wvsc_d = nc.dram_tensor("wv_sc", [128, 27], F32,
                            kind="ExternalInput").ap()
    wp2_d = nc.dram_tensor("wp2", [128, 128], F32R, kind="ExternalInput").ap()
    tvec_d = nc.dram_tensor("tvec", [128, 1], F32, kind="ExternalInput").ap()
    bdmask_d = nc.dram_tensor("bdmask", [128, 128], F32, kind="ExternalInput").ap()
    ident_d = nc.dram_tensor("ident", [128, 128], F32, kind="ExternalInput").ap()
    identr_d = nc.dram_tensor("ident_r", [128, 128], F32R, kind="ExternalInput").ap()
    ones_d = nc.dram_tensor("ones_row", [1, 128], F32R, kind="ExternalInput").ap()
    zeros_d = nc.dram_tensor("zeros128", [128, 128], F32R, kind="ExternalInput").ap()

    out_d = nc.dram_tensor("out_sh", [128, MOUT], F32, kind="ExternalOutput").ap()
    dbg_g = nc.dram_tensor("dbg_g", [128, 256], F32, kind="ExternalOutput").ap()
    dbg_ab = nc.dram_tensor("dbg_ab", [128, 128], F32R, kind="ExternalOutput").ap()

    with tile.TileContext(nc) as tc:
        _emit(nc, tc, x_d, w2_d, wdv_d, wqp_d, wqs_d, id8_d, wvsc_d, wp2_d,
              tvec_d, bdmask_d, ident_d, identr_d, ones_d, zeros_d, out_d,
              dbg_g, dbg_ab)
    nc.compile()
    return nc


def _emit(nc, tc, x_d, w2_d, wdv_d, wqp_d, wqs_d, id8_d, wvsc_d, wp2_d,
          tvec_d, bdmask_d, ident_d, identr_d, ones_d, zeros_d, out_d,
          dbg_g, dbg_ab):
    from contextlib import ExitStack
    es = ExitStack()

    cons = es.enter_context(tc.tile_pool(name="cons", bufs=1))
    xp = es.enter_context(tc.tile_pool(name="xp", bufs=3))
    qkvp = es.enter_context(tc.tile_pool(name="qkvp", bufs=4))
    vslp = es.enter_context(tc.tile_pool(name="vslp", bufs=5))
    vaccp = es.enter_context(tc.tile_pool(name="vaccp", bufs=3))
    vp = es.enter_context(tc.tile_pool(name="vp", bufs=1))
    stp = es.enter_context(tc.tile_pool(name="stp", bufs=3))
    qtp = es.enter_context(tc.tile_pool(name="qtp", bufs=3))
    gsp = es.enter_context(tc.tile_pool(name="gsp", bufs=1))
    smp = es.enter_context(tc.tile_pool(name="smp", bufs=1))
    outp = es.enter_context(tc.tile_pool(name="outp", bufs=2))
    dramp = es.enter_context(tc.tile_pool(name="dramp", bufs=1, space="DRAM"))

    pw_ps = es.enter_context(tc.tile_pool(name="pw_ps", bufs=2, space="PSUM"))
    dw_ps = es.enter_context(tc.tile_pool(name="dw_ps", bufs=2, space="PSUM"))
    tr_ps = es.enter_context(tc.tile_pool(name="tr_ps", bufs=2, space="PSUM"))
    gr_ps = es.enter_context(tc.tile_pool(name="gr_ps", bufs=2, space="PSUM"))

    # ---- constants ----
    w2s = cons.tile([128, 384], F32R, tag="w2s")
    nc.sync.dma_start(w2s[:], w2_d[:])
    wdvs = cons.tile([128, 27 * 128], FP16, tag="wdvs")
    wdvv = wdvs[:].rearrange("p (t j) -> p t j", t=27, j=128)
    nc.sync.dma_start(wdvv, wdv_d.rearrange("t i j -> i t j"))
    wqp = cons.tile([128, 12 * 2 * 128], FP8, tag="wqp")
    wqpv = wqp[:].rearrange("p (pr i j) -> p pr i j", pr=12, i=2, j=128)
    nc.sync.dma_start(wqpv, wqp_d.rearrange("pr i a j -> i pr a j"))
    wqs = cons.tile([128, 3 * 128], FP8, tag="wqs")
    wqsv = wqs[:].rearrange("p (t j) -> p t j", t=3, j=128)
    nc.sync.dma_start(wqsv, wqs_d.rearrange("t i j -> i t j"))
    wvs = cons.tile([128, 27], F32, tag="wvs")
    nc.sync.dma_start(wvs[:], wvsc_d[:])
    idb = cons.tile([128, 128], BF16, tag="idb")
    from concourse.masks import make_identity
    make_identity(nc, idb[:])
    wp2s = cons.tile([128, 128], F32R, tag="wp2s")
    nc.sync.dma_start(wp2s[:], wp2_d[:])
    tvs = cons.tile([128, 1], F32, tag="tvs")
    nc.sync.dma_start(tvs[:], tvec_d[:])
    bds = cons.tile([128, 128], F32, tag="bds")
    nc.sync.dma_start(bds[:], bdmask_d[:])
    ids = cons.tile([128, 128], F32, tag="ids")
    nc.sync.dma_start(ids[:], ident_d[:])
    idr = cons.tile([128, 128], F32R, tag="idr")
    nc.sync.dma_start(idr[:], identr_d[:])
    on1 = cons.tile([1, 128], F32R, tag="on1")
    nc.sync.dma_start(on1[:], ones_d[:])
    zqv = cons.tile([128, MP], FP16, tag="zqv")
    nc.gpsimd.memset(zqv[:], 0.0)
    zqk = cons.tile([128, MP], FP8, tag="zqk")
    nc.gpsimd.memset(zqk[:], 0.0)

    g_sb = []
    for b in range(2):
        g = gsp.tile([128, 128], F32, tag=f"g{b}")
        nc.vector.memset(g[:], 0.0)
        g_sb.append(g)

    vres = vp.tile([128, MOUT], F32R, tag="vres")

    ev_ctr = [0]

    def evac(dst, src):
        if ev_ctr[0] % 2 == 0:
            nc.vector.tensor_copy(dst, src)
        else:
            nc.scalar.copy(dst, src)
        ev_ctr[0] += 1

    # ---- phase 1: pointwise -> depthwise -> gram, software-pipelined over
    # d: step s runs pointwise(s), qk-depthwise(s-1), v-depthwise(s-2). The
    # two trailing v iterations keep the PE busy under the gram AllReduce.
    qk_slots = [None] * D
    v_slots = [None] * D

    def tap_ap(tile_ap, offset, dims):
        a = tile_ap.copy()
        pstride = list(a.ap)[0][0]
        a.ap = mybir.VecI64Pair([[pstride, 128]] + dims)
        a.offset = offset
        return a

    def pointwise(d):
        xs = xp.tile([128, MS], F32R, tag="xs")
        nc.sync.dma_start(xs[:], x_d[:, d])
        qks = qkvp.tile([128, 2 * MP], FP8, tag="qk")
        vs = vslp.tile([128, MP], FP16, tag="vsl")
        qk_slots[d] = qks
        v_slots[d] = vs
        qksv = qks[:].rearrange("p (beta hh ww) -> p beta hh ww",
                                beta=2, hh=HLH, ww=WP)
        vsv = vs[:].rearrange("p (hh ww) -> p hh ww", hh=HLH, ww=WP)
        # zero padded w-border columns (slots rotate; memory is dirty)
        nc.gpsimd.memset(qksv[:, :, :, 0:WP:WP - 1], 0.0)
        nc.gpsimd.memset(vsv[:, :, 0:WP:WP - 1], 0.0)
        for beta in range(3):
            for t6 in range(6):
                ps = pw_ps.tile([128, 384], F32, tag="pw")
                nc.tensor.matmul(
                    ps[:],
                    w2s[:, 128 * beta:128 * (beta + 1)],
                    xs[:, 384 * t6:384 * (t6 + 1)],
                    start=True, stop=True)
                if beta < 2:
                    dst = qksv[:, beta, 3 * t6:3 * t6 + 3, 1:1 + W]
                else:
                    dst = vsv[:, 3 * t6:3 * t6 + 3, 1:1 + W]
                evac(dst, ps[:])

    # per-kd tap pairing: j = kh*3+kw; pairs (0,1),(2,3),(4,5),(6,7), single 8
    PAIR_J0 = [0, 2, 4, 6]
    J_OFF = [kh * WP + kw for kh in range(3) for kw in range(3)]

    def qk_dw(do):
        for beta in range(2):
            gp = gr_ps.tile([128, 128], F32, tag="gram")
            n_gmm = 0
            for t4 in range(4):
                dps = dw_ps.tile([128, 512], F32, tag="dw")
                mm = 0
                for kd in range(3):
                    dd = do - 1 + kd
                    slot = qk_slots[dd] if 0 <= dd < D else None
                    base = (beta * MP if slot is not None else 0) + 4 * t4 * WP
                    src = slot if slot is not None else zqk
                    for j0 in PAIR_J0:
                        delta = J_OFF[j0 + 1] - J_OFF[j0]
                        rhs = tap_ap(src[:], base + J_OFF[j0],
                                     [[delta, 2], [WP, 4], [1, W]])
                        nc.tensor.matmul(
                            dps[:], wqpv[:, 4 * kd + j0 // 2], rhs,
                            start=(mm == 0), stop=False,
                            perf_mode=mybir.MatmulPerfMode.DoubleRow)
                        mm += 1
                    rhs = tap_ap(src[:], base + J_OFF[8],
                                 [[WP, 4], [1, W]])
                    nc.tensor.matmul(dps[:], wqsv[:, kd], rhs,
                                     start=False, stop=(kd == 2))
                    mm += 1
                st = stp.tile([128, 512], BF16, tag="st")
                evac(st[:], dps[:])
                for ch4 in range(4):
                    trp = tr_ps.tile([128, 128], BF16, tag="tr")
                    nc.tensor.transpose(
                        trp[:], st[:, 128 * ch4:128 * (ch4 + 1)], idb[:])
                    qt = qtp.tile([128, 128], BF16, tag="qt")
                    evac(qt[:], trp[:])
                    nc.tensor.matmul(gp[:], qt[:], qt[:],
                                     start=(n_gmm == 0), stop=(n_gmm == 15))
                    n_gmm += 1
            nc.vector.tensor_add(g_sb[beta][:], g_sb[beta][:], gp[:])

    def v_dw(do):
        for t4 in range(4):
            dps = dw_ps.tile([128, 512], F32, tag="dw")
            for t, (kd, kh, kw) in enumerate(TAPS):
                dd = do - 1 + kd
                src = v_slots[dd] if 0 <= dd < D else zqv
                sv = src[:].rearrange("p (hh ww) -> p hh ww", hh=HLH, ww=WP)
                rhs = sv[:, 4 * t4 + kh:4 * t4 + kh + 4, kw:kw + W]
                nc.tensor.matmul(dps[:], wdvv[:, t], rhs,
                                 start=(t == 0), stop=(t == 26))
            evac(vres[:, 2048 * do + 512 * t4:2048 * do + 512 * (t4 + 1)],
                 dps[:])

    for step in range(D + 3):
        if step < D:
            pointwise(step)
        if 0 <= step - 1 < D:
            qk_dw(step - 1)
        if 0 <= step - 3 < D:
            v_dw(step - 3)

    # ---- all-reduce the grams ----
    bnc_in = dramp.tile([128, 256], F32, tag="bnc_in")
    bnc_out = dramp.tile([128, 256], F32, tag="bnc_out", addr_space="Shared")
    nc.gpsimd.dma_start(bnc_in[:, 0:128], g_sb[0][:])
    nc.gpsimd.dma_start(bnc_in[:, 128:256], g_sb[1][:])
    nc.gpsimd.collective_compute(
        "AllReduce", mybir.AluOpType.add,
        replica_groups=[list(range(N_CORES))],
        ins=[bnc_in.opt()], outs=[bnc_out.opt()])
    nc.gpsimd.dma_start(g_sb[0][:], bnc_out[:, 0:128])
    nc.gpsimd.dma_start(g_sb[1][:], bnc_out[:, 128:256])
    nc.sync.dma_start(dbg_g[:, 0:128], g_sb[0][:])
    nc.sync.dma_start(dbg_g[:, 128:256], g_sb[1][:])

    # ---- softmax -> attention weights AB (block-diag per batch) ----
    ab = smp.tile([128, 128], F32R, tag="ab")
    nc.sync.dma_start(ab[:], zeros_d[:])
    for b in range(2):
        gb = g_sb[b]
        dtmp = smp.tile([128, 128], F32, tag=f"dtmp{b}")
        nc.vector.tensor_mul(dtmp[:], gb[:], ids[:])
        dvec = smp.tile([128, 1], F32, tag=f"dvec{b}")
        nc.vector.reduce_sum(dvec[:], dtmp[:], axis=mybir.AxisListType.X)
        nrm = smp.tile([128, 1], F32, tag=f"nrm{b}")
        nc.scalar.activation(nrm[:], dvec[:], mybir.ActivationFunctionType.Sqrt)
        nc.vector.tensor_scalar_max(nrm[:], nrm[:], EPS)
        rn = smp.tile([128, 1], F32, tag=f"rn{b}")
        nc.vector.reciprocal(rn[:], nrm[:])
        # RK[p, j] = rn[j] via transpose + K=1 outer product
        rtp = tr_ps.tile([1, 128], F32, tag="tr")
        nc.tensor.transpose(rtp[:], rn[:], ids[:])
        rnt_row = smp.tile([1, 128], F32R, tag=f"rnt_row{b}")
        nc.vector.tensor_copy(rnt_row[:], rtp[:])
        rkp = gr_ps.tile([128, 128], F32, tag="gram")
        nc.tensor.matmul(rkp[:], on1[:], rnt_row[:],
                         start=True, stop=True)
        rk = smp.tile([128, 128], F32, tag=f"rk{b}")
        nc.vector.tensor_copy(rk[:], rkp[:])
        # s = G * (rn*temp)[row] * rn[col]
        rnt = smp.tile([128, 1], F32, tag=f"rnt{b}")
        nc.vector.tensor_mul(rnt[:], rn[:], tvs[:])
        s1 = smp.tile([128, 128], F32, tag=f"s1{b}")
        nc.vector.tensor_scalar_mul(s1[:], gb[:], rnt[:])
        nc.vector.tensor_mul(s1[:], s1[:], rk[:])
        e = smp.tile([128, 128], F32, tag=f"e{b}")
        nc.scalar.activation(e[:], s1[:], mybir.ActivationFunctionType.Exp)
        nc.vector.tensor_mul(e[:], e[:], bds[:])
        ssum = smp.tile([128, 1], F32, tag=f"ssum{b}")
        nc.vector.reduce_sum(ssum[:], e[:, 64:128], axis=mybir.AxisListType.X)
        nc.vector.tensor_scalar_max(ssum[:], ssum[:], 1e-30)
        rs = smp.tile([128, 1], F32, tag=f"rs{b}")
        nc.vector.reciprocal(rs[:], ssum[:])
        apre = smp.tile([128, 128], F32, tag=f"apre{b}")
        nc.vector.tensor_scalar_mul(apre[:], e[:], rs[:])
        # transpose the q-k quadrant into the (b,b) diagonal block of AB
        abp = tr_ps.tile([64, 64], F32, tag="tr")
        nc.tensor.transpose(abp[:], apre[0:64, 64:128], ids[0:64, 0:64])
        if b == 0:
            nc.vector.tensor_copy(ab[0:64, 0:64], abp[:])
        else:
            absb = smp.tile([64, 64], F32R, tag="absb")
            nc.vector.tensor_copy(absb[:], abp[:])
            nc.sync.dma_start(ab[64:128, 64:128], absb[:])
    nc.sync.dma_start(dbg_ab[:], ab[:])

    # ---- attn @ v -> projection -> out ----
    for mt in range(MOUT // 512):
        sl = slice(512 * mt, 512 * (mt + 1))
        avp = dw_ps.tile([128, 512], F32, tag="dw")
        nc.tensor.matmul(avp[:], ab[:], vres[:, sl],
                         start=True, stop=True)
        avs = stp.tile([128, 512], F32R, tag="st")
        evac(avs[:], avp[:])
        prp = dw_ps.tile([128, 512], F32, tag="dw")
        nc.tensor.matmul(prp[:], wp2s[:], avs[:],
                         start=True, stop=True)
        ots = outp.tile([128, 512], F32, tag="ots")
        evac(ots[:], prp[:])
        nc.sync.dma_start(out_d[:, sl], ots[:])

    es.close()


def _prep_in_maps(x, w_qkv, w_dw, temperature, w_proj):
    consts = _prep_consts(w_qkv, w_dw, temperature, w_proj)
    shards = _prep_x_shards(x)
    in_maps = []
    for r in range(N_CORES):
        in_maps.append({
            "x_sh": shards[r],
            "w2": consts["w2"],
            "wdv": consts["wdv"],
            "wdqk_pr": consts["wdqk_pr"],
            "wdqk_sg": consts["wdqk_sg"],
            "ident8": consts["ident8"],
            "wv_sc": consts["wv_sc"],
            "wp2": consts["wp2"],
            "tvec": consts["tvec"],
            "bdmask": consts["bdmask"],
            "ident": consts["ident"],
            "ident_r": consts["ident"],
            "ones_row": consts["ones_row"],
            "zeros128": np.zeros((128, 128), np.float32),
        })
    return in_maps


def _unshard(res):
    out = np.empty((B, C, D, H, W), np.float32)
    for r in range(N_CORES):
        slab = res.results[r]["out_sh"].reshape(B, C, D, HL, W)
        out[:, :, :, HL * r:HL * (r + 1), :] = slab
    return out


def kernel(x, w_qkv, w_dw, temperature, w_proj):
    if "nc" not in _CACHE:
        _CACHE["nc"] = _build_program()
    in_maps = _prep_in_maps(x, w_qkv, w_dw, temperature, w_proj)
    res = bass_utils.run_bass_kernel_spmd(
        _CACHE["nc"], in_maps, core_ids=list(range(N_CORES)))
    _CACHE["last_res"] = res
    return _unshard(res)


def run_profiled(x, w_qkv, w_dw, temperature, w_proj, **trace_kw):
    if "nc" not in _CACHE:
        _CACHE["nc"] = _build_program()
    in_maps = _prep_in_maps(x, w_qkv, w_dw, temperature, w_proj)
    res = bass_utils.run_bass_kernel_spmd(
        _CACHE["nc"], in_maps, core_ids=list(range(N_CORES)),
        trace=True, trace_cores=list(range(N_CORES)), **trace_kw)
    _CACHE["last_res"] = res
    return res


if __name__ == "__main__":
    rng = np.random.default_rng(0)
    x = rng.standard_normal((B, C, D, H, W), dtype=np.float32)
    w_qkv = rng.standard_normal((C3, C), dtype=np.float32) * 0.05
    w_dw = rng.standard_normal((C3, 1, 3, 3, 3), dtype=np.float32) * 0.05
    temperature = np.ones((HEADS, 1, 1), np.float32)
    w_proj = rng.standard_normal((C, C), dtype=np.float32) * 0.05
    out = kernel(x=x, w_qkv=w_qkv, w_dw=w_dw, temperature=temperature,
                 w_proj=w_proj)
    print("out", out.shape, out.dtype, np.abs(out).mean())
